# revision 1
# baseline (speedup 1.0000x reference)
"""Causal single-head attention (B=4, S=4096, D=2048) on 8 trn2 NeuronCores.

Sharding: core = (batch b, query-half h). Query blocks of 128 rows are
interleaved between the two halves ({4j,4j+3} vs {4j+1,4j+2} within each
group of 4) so that both halves execute an identical instruction stream
(SPMD) with balanced causal work. Per core: 8 strips of 256 queries;
strip j attends to keys [0, 512*(j+1)).

v3 = v1 (zero-collective design) + pipeline fixes:
  - x arrives pre-transposed (d-major) from the host: plain contiguous
    DMA everywhere, no dma_start_transpose.
  - Each core computes K^T for ALL 4096 keys of its batch locally.
  - V is never materialized: out = (P @ x) @ Wv (Z^T accumulated d-major,
    then a Wv projection replaces the V projection at equal FLOPs).
  - One shared pool set across the K and Q projections (no pool-boundary
    barrier); W streamed as [128,512] quarter-column tiles with 2-deep
    rotation on the scalar DMA queue while x-tiles ride the sync queue.
  - kg/qs pools hoisted above the projections so attention prefetch
    overlaps the Q projection.
"""

import sys

try:
    import concourse  # noqa: F401
except ImportError:
    sys.path.insert(0, "/opt/trn_rl_repo")

import numpy as np
import ml_dtypes

import concourse.bass as bass
import concourse.mybir as mybir
import concourse.tile as tile
from concourse import bacc
from concourse.bass_utils import run_bass_kernel_spmd

B, S, D = 4, 4096, 2048
NQ = S // 2          # queries per core
C = D // 128         # 16 contraction chunks
STRIPS = 8           # strips of 256 queries per core
SQ = NQ // STRIPS    # 256
SCALE = 1.0 / float(np.sqrt(D))

BF = mybir.dt.bfloat16
F32 = mybir.dt.float32


def _blocks_for_half(h: int) -> list[int]:
    # strip-major order; strip j covers global blocks {4j+0,4j+3} or {4j+1,4j+2}
    off = (0, 3) if h == 0 else (1, 2)
    return [4 * j + o for j in range(STRIPS) for o in off]


def build_nc(variant="full", reps=1):
    nc = bacc.Bacc("TRN2", target_bir_lowering=False, debug=False, num_devices=8)

    # d-major tiled inputs: [d-chunk, d-in-chunk, seq]
    xqT = nc.dram_tensor("xqT", [C, 128, NQ], BF, kind="ExternalInput")
    xkT = nc.dram_tensor("xkT", [C, 128, S], BF, kind="ExternalInput")
    # natural-layout x of the full batch (keys-major) for the AV matmul
    xn = nc.dram_tensor("xn", [S, D], BF, kind="ExternalInput")
    wq = nc.dram_tensor("Wq", [D, D], BF, kind="ExternalInput")
    wk = nc.dram_tensor("Wk", [D, D], BF, kind="ExternalInput")
    wv = nc.dram_tensor("Wv", [D, D], BF, kind="ExternalInput")
    # maskT[512*j + kk, qq]: multiplicative mask for strip j's diagonal key
    # group, key-major (matches the transposed score layout)
    maskT = nc.dram_tensor("maskT", [S, SQ], BF, kind="ExternalInput")
    out = nc.dram_tensor("out", [NQ, D], F32, kind="ExternalOutput")

    kT = nc.dram_tensor("kT", [C, 128, S], BF, kind="Internal")
    qT = nc.dram_tensor("qT", [C, 128, NQ], BF, kind="Internal")

    with tile.TileContext(nc) as tc:
        for _rep in range(reps):
                _emit(nc, tc, xqT, xkT, xn, wq, wk, wv, maskT, out, kT, qT, variant)

    nc.compile()
    return nc


def _emit(nc, tc, xqT, xkT, xn, wq, wk, wv, maskT, out, kT, qT, variant="full"):

    if variant == "cast":
        with tc.tile_pool(name="dummy", bufs=1) as dp:
            z = dp.tile([128, 1024], F32)
            nc.vector.memset(z[:], 0.0)
            for r in range(NQ // 128):
                for hh in range(2):
                    nc.sync.dma_start(
                        out=out.ap()[128 * r : 128 * (r + 1), 1024 * hh : 1024 * (hh + 1)],
                        in_=z[:],
                    )
        return

    # kg/qs pools hoisted: their prefetch DMAs overlap the projections.
    with (
        tc.tile_pool(name="qs", bufs=20) as qsp,
        tc.tile_pool(name="kg", bufs=24) as kgp,
    ):
        # ---- Projections: outT[c, :, s] = (x @ W)^T (d-major). W streamed
        # as [128, 512] m-quarter tiles (bufs=2 rotation, scalar DMA queue);
        # stationary tile reused across SB seq-tiles so LDWEIGHTS amortizes.
        SB = 2  # seq-tiles per block
        with (
            tc.tile_pool(name="w", bufs=2) as wp,
            tc.tile_pool(name="xt", bufs=36) as xtp,
            tc.tile_pool(name="pps", bufs=8, space="PSUM") as pps,
            tc.tile_pool(name="pcp", bufs=8) as pcp,
        ):
            def project_dmajor(w_dram, xT_dram, n_rows, outT):
                for sb in range(n_rows // 512 // SB):
                    xt = {}
                    for s4 in range(SB):
                        s = SB * sb + s4
                        for c in range(C):
                            t = xtp.tile([128, 512], BF, name="xt")
                            nc.sync.dma_start(
                                out=t[:],
                                in_=xT_dram.ap()[c, :, 512 * s : 512 * (s + 1)],
                            )
                            xt[(s4, c)] = t
                    for qtr in range(4):
                        w_q = []
                        for c in range(C):
                            t = wp.tile([128, 512], BF, name=f"wq{c}")
                            nc.scalar.dma_start(
                                out=t[:],
                                in_=w_dram.ap()[
                                    128 * c : 128 * (c + 1),
                                    512 * qtr : 512 * (qtr + 1),
                                ],
                            )
                            w_q.append(t)
                        for mi in range(4):
                            m = 4 * qtr + mi
                            ps = [
                                pps.tile([128, 512], F32, name="pps_t")
                                for _ in range(SB)
                            ]
                            for c in range(C):
                                for s4 in range(SB):
                                    nc.tensor.matmul(
                                        ps[s4][:],
                                        lhsT=w_q[c][:, 128 * mi : 128 * (mi + 1)],
                                        rhs=xt[(s4, c)][:],
                                        start=(c == 0), stop=(c == C - 1),
                                    )
                            for s4 in range(SB):
                                s = SB * sb + s4
                                o = pcp.tile([128, 512], BF, name="pcp_t")
                                nc.scalar.copy(o[:], ps[s4][:])
                                nc.scalar.dma_start(
                                    out=outT.ap()[m, :, 512 * s : 512 * (s + 1)],
                                    in_=o[:],
                                )

            project_dmajor(wk, xkT, S, kT)    # K^T for all 4096 keys (local)
            project_dmajor(wq, xqT, NQ, qT)   # Q^T for this core's queries

        if variant == "proj":
            with tc.tile_pool(name="drain", bufs=4) as dp:
                for r in range(NQ // 128):
                    z = dp.tile([128, D], F32, name="drain_t")
                    zk = dp.tile([128, D], BF, name="drain_k")
                    nc.sync.dma_start(out=zk[:, 0:256], in_=kT.ap()[r % C, :, 0:256])
                    nc.sync.dma_start(out=zk[:, 256:512], in_=qT.ap()[r % C, :, 0:256])
                    nc.vector.tensor_copy(z[:], zk[:])
                    nc.sync.dma_start(out=out.ap()[128 * r : 128 * (r + 1), :], in_=z[:])
            return

        # ---- Attention, strip-pair by strip-pair ----
        # Pair p covers strips 2p (queries [512p, 512p+256), key bound
        # 512(2p+1)) and 2p+1 (queries [512p+256, 512p+512), bound
        # 512(2p+2)). Scores run pair-wide (N=512) except the last key
        # group (odd member only, N=256). Z^T accumulates in 4 d-quarter
        # passes; the Wv projection then runs per 512-e pass.
        with (
            tc.tile_pool(name="ones", bufs=1) as onesp,
            tc.tile_pool(name="wv", bufs=1) as wvp,
            tc.tile_pool(name="pt", bufs=36) as ptp,
            tc.tile_pool(name="xg", bufs=8) as xgp,
            tc.tile_pool(name="zt", bufs=20) as ztp,
            tc.tile_pool(name="msk", bufs=8) as mskp,
            tc.tile_pool(name="rcp", bufs=8) as rcpp,
            tc.tile_pool(name="osb", bufs=6) as osbp,
            tc.tile_pool(name="ps_s", bufs=2, space="PSUM") as ps_s,
            tc.tile_pool(name="ps_zo", bufs=4, space="PSUM") as ps_zo,
            tc.tile_pool(name="ps_d", bufs=1, space="PSUM") as ps_d,
        ):
            ones = onesp.tile([128, 1], BF)
            nc.vector.memset(ones[:], 1.0)
            idf = onesp.tile([1, 1], F32, name="idf")
            nc.vector.memset(idf[:], 1.0)

            wv_sb = []
            for c in range(C):
                t = wvp.tile([128, D], BF, name=f"wv_sb{c}")
                nc.sync.dma_start(out=t[:], in_=wv.ap()[128 * c : 128 * (c + 1), :])
                wv_sb.append(t)

            NPAIR = STRIPS // 2
            for i in range(NPAIR):
                ng_even = 2 * i + 1   # groups for subs 0,1 (strip 2i)
                ng_odd = 2 * i + 2    # groups for subs 2,3 (strip 2i+1)
                qs = []
                for c in range(C):
                    t = qsp.tile([128, 512], BF, name="qs_t")
                    nc.sync.dma_start(
                        out=t[:], in_=qT.ap()[c, :, 512 * i : 512 * (i + 1)]
                    )
                    qs.append(t)

                # Phase A: P^T chunks
                pt = []
                co_of = {}
                for g in range(ng_odd):
                    full_pair = g < ng_even  # last group: odd member only
                    kg = []
                    for c in range(C):
                        t = kgp.tile([128, 512], BF, name="kg_t")
                        nc.sync.dma_start(
                            out=t[:], in_=kT.ap()[c, :, 512 * g : 512 * (g + 1)]
                        )
                        kg.append(t)
                    for kk in range(4):
                        # column start: diagonal-group chunks whose low
                        # columns are fully causal-masked for BOTH halves
                        # are skipped (those pt regions are memset to 0).
                        if g == 2 * i:        # diag of strip 2i
                            co = 0 if kk < 2 else 128
                        elif not full_pair:   # last group: diag of 2i+1
                            co = 256 if kk < 2 else 384
                        else:
                            co = 0
                        ps = ps_s.tile([128, 512], F32, name="ps_s_t")
                        for c in range(C):
                            nc.tensor.matmul(
                                ps[:, co:512],
                                lhsT=kg[c][:, 128 * kk : 128 * (kk + 1)],
                                rhs=qs[c][:, co:512],
                                start=(c == 0), stop=(c == C - 1),
                            )
                        p = ptp.tile([128, 512], BF, name="pt_t")
                        if co > 0:
                            nc.vector.memset(p[:, 0:co], 0.0)
                        nc.scalar.activation(
                            out=p[:, co:512], in_=ps[:, co:512],
                            func=mybir.ActivationFunctionType.Exp, scale=SCALE,
                        )
                        co_of[4 * g + kk] = co
                        # diagonal-group masks, per member strip
                        for member, js in ((0, 2 * i), (1, 2 * i + 1)):
                            if g == js:
                                mk = mskp.tile([128, SQ], BF, name="msk_t")
                                r0 = 512 * js + 128 * kk
                                nc.sync.dma_start(
                                    out=mk[:], in_=maskT.ap()[r0 : r0 + 128, :]
                                )
                                cols = slice(256 * member, 256 * (member + 1))
                                nc.vector.tensor_mul(p[:, cols], p[:, cols], mk[:])
                        pt.append(p)

                # Phase B: denominators. ones is the STATIONARY operand
                # (no per-chunk LDWEIGHTS); dn1[0, q] accumulates column
                # sums of P^T over all key chunks (the causally-invalid pt
                # regions are zero), then 4 PE transposes turn the [1,512]
                # row into per-sub [128,1] scalars.
                dn1 = ps_d.tile([1, 512], F32, name="dn1_t")
                for g in range(ng_odd):
                    for kk in range(4):
                        nc.tensor.matmul(
                            dn1[:], lhsT=ones[:, 0:1], rhs=pt[4 * g + kk][:],
                            start=(g == 0 and kk == 0),
                            stop=(g == ng_odd - 1 and kk == 3),
                        )
                dn_sb = rcpp.tile([1, 512], F32, name="dnsb_t")
                nc.scalar.copy(dn_sb[:], dn1[:])
                rec_sb = [None] * 4
                for u in range(4):
                    dt_ps = ps_d.tile([128, 1], F32, name="dnT_t")
                    nc.tensor.transpose(
                        dt_ps[:], dn_sb[0:1, 128 * u : 128 * (u + 1)],
                        idf[0:1, 0:1],
                    )
                    r = rcpp.tile([128, 1], F32, name="rec_t")
                    nc.vector.reciprocal(r[:], dt_ps[:])
                    rec_sb[u] = r

                # Phase B': Z^T = sum_k x[k,:]^T P^T[k,:] in 4 d-quarter
                # passes. For the last key group only query cols 256:512 are
                # valid, so those matmuls accumulate into the right half.
                zt = {}
                for qp in range(4):
                    z_ps = [
                        ps_zo.tile([128, 512], F32, name="zo_ps") for _ in range(4)
                    ]
                    for g in range(ng_odd):
                        full_pair = g < ng_even
                        for kk in range(4):
                            kc = 4 * g + kk
                            r0 = 512 * g + 128 * kk
                            vt = xgp.tile([128, 512], BF, name="xg_t")
                            nc.scalar.dma_start(
                                out=vt[:],
                                in_=xn.ap()[r0 : r0 + 128, 512 * qp : 512 * (qp + 1)],
                            )
                            co = co_of[kc]
                            for c4 in range(4):
                                first = g == 0 and kk == 0
                                last = g == ng_odd - 1 and kk == 3
                                nc.tensor.matmul(
                                    z_ps[c4][:, co:512],
                                    lhsT=vt[:, 128 * c4 : 128 * (c4 + 1)],
                                    rhs=pt[kc][:, co:512],
                                    start=first, stop=last,
                                )
                    for c4 in range(4):
                        zt_t = ztp.tile([128, 512], BF, name="zt_t")
                        nc.scalar.copy(zt_t[:], z_ps[c4][:])
                        zt[4 * qp + c4] = zt_t

                # Phase C: out = Z @ Wv, then normalize by 1/den and store.
                for ep in range(4):
                    o_ps = [
                        ps_zo.tile([128, 512], F32, name="zo_ps") for _ in range(4)
                    ]
                    for u in range(4):
                        for c in range(C):
                            nc.tensor.matmul(
                                o_ps[u][:],
                                lhsT=zt[c][:, 128 * u : 128 * (u + 1)],
                                rhs=wv_sb[c][:, 512 * ep : 512 * (ep + 1)],
                                start=(c == 0), stop=(c == C - 1),
                            )
                    for u in range(4):
                        o = osbp.tile([128, 512], F32, name="osb_t")
                        nc.vector.tensor_scalar_mul(o[:], o_ps[u][:], rec_sb[u][:])
                        r0 = 512 * i + 128 * u
                        nc.sync.dma_start(
                            out=out.ap()[r0 : r0 + 128, 512 * ep : 512 * (ep + 1)],
                            in_=o[:],
                        )


_NC_CACHE = {}


def _get_nc(variant="full", reps=1):
    key = (variant, reps)
    if key not in _NC_CACHE:
        _NC_CACHE[key] = build_nc(variant, reps)
    return _NC_CACHE[key]


def _dmajor_tiles(xt: np.ndarray) -> np.ndarray:
    """[rows, D] -> [C, 128, rows] (d-major tiled), bf16."""
    return np.ascontiguousarray(xt.T.reshape(C, 128, -1))


def _core_inputs(x, Wq, Wk, Wv, b, h):
    blocks = _blocks_for_half(h)
    qpos = (128 * np.asarray(blocks)[:, None] + np.arange(128)[None, :]).reshape(-1)
    xb = np.asarray(x[b], dtype=ml_dtypes.bfloat16)
    xq = xb[qpos]
    maskT = np.zeros((S, SQ), dtype=np.float32)
    for j in range(STRIPS):
        keys = 512 * j + np.arange(512)[:, None]
        qp = qpos[SQ * j : SQ * (j + 1)][None, :]
        maskT[512 * j : 512 * (j + 1), :] = (keys <= qp).astype(np.float32)
    return {
        "xqT": _dmajor_tiles(xq),
        "xkT": _dmajor_tiles(xb),
        "xn": np.ascontiguousarray(xb),
        "Wq": np.ascontiguousarray(Wq).astype(ml_dtypes.bfloat16),
        "Wk": np.ascontiguousarray(Wk).astype(ml_dtypes.bfloat16),
        "Wv": np.ascontiguousarray(Wv).astype(ml_dtypes.bfloat16),
        "maskT": maskT.astype(ml_dtypes.bfloat16),
    }, qpos


def kernel(x, Wq, Wk, Wv, _want_results=False):
    x = np.asarray(x)
    Wq, Wk, Wv = np.asarray(Wq), np.asarray(Wk), np.asarray(Wv)
    nc = _get_nc()

    in_maps, qposes = [], []
    for b in range(B):
        for h in range(2):
            im, qpos = _core_inputs(x, Wq, Wk, Wv, b, h)
            in_maps.append(im)
            qposes.append((b, qpos))

    res = run_bass_kernel_spmd(nc, in_maps, core_ids=list(range(8)))

    out = np.empty((B, S, D), dtype=np.float32)
    for core, (b, qpos) in enumerate(qposes):
        out[b][qpos] = res.results[core]["out"]
    if _want_results:
        return out, res
    return out


def measure_exec_ns(inputs, iters=48, variant="full"):
    """Estimate per-launch device execution time by pipelining `iters`
    dispatches of the compiled executable with device-resident inputs
    (amortizes host/tunnel dispatch overhead); returns marginal ns/exec."""
    import time
    import jax
    from jax.sharding import Mesh, PartitionSpec, NamedSharding
    from jax.experimental.shard_map import shard_map
    from concourse.bass2jax import (
        _bass_exec_p, install_neuronx_cc_hook, partition_id_tensor,
    )

    nc = _get_nc(variant)
    install_neuronx_cc_hook()
    in_names, out_names, out_avals, zero_outs = [], [], [], []
    for alloc in nc.m.functions[0].allocations:
        if not isinstance(alloc, mybir.MemoryLocationSet):
            continue
        name = alloc.memorylocations[0].name
        if alloc.kind == "ExternalInput":
            if nc.partition_id_tensor is None or name != nc.partition_id_tensor.name:
                in_names.append(name)
        elif alloc.kind == "ExternalOutput":
            out_names.append(name)
            shape = tuple(alloc.tensor_shape)
            dtype = mybir.dt.np(alloc.dtype)
            out_avals.append(jax.core.ShapedArray(shape, dtype))
            zero_outs.append(np.zeros(shape, dtype))
    n_params = len(in_names)
    n_outs = len(out_avals)
    all_names = in_names + out_names
    if nc.partition_id_tensor is not None:
        all_names = all_names + [nc.partition_id_tensor.name]

    def _body(*args):
        operands = list(args)
        if nc.partition_id_tensor is not None:
            operands.append(partition_id_tensor())
        return tuple(_bass_exec_p.bind(
            *operands, out_avals=tuple(out_avals), in_names=tuple(all_names),
            out_names=tuple(out_names), lowering_input_output_aliases=(),
            sim_require_finite=True, sim_require_nnan=True, nc=nc,
        ))

    devices = jax.devices()[:8]
    mesh = Mesh(np.array(devices), ("core",))
    sharded = jax.jit(
        shard_map(_body, mesh=mesh,
                  in_specs=(PartitionSpec("core"),) * (n_params + n_outs),
                  out_specs=(PartitionSpec("core"),) * n_outs,
                  check_rep=False),
        donate_argnums=tuple(range(n_params, n_params + n_outs)),
        keep_unused=True,
    )
    in_maps = []
    x, Wq, Wk, Wv = inputs["x"], inputs["Wq"], inputs["Wk"], inputs["Wv"]
    for b in range(B):
        for h in range(2):
            im, _ = _core_inputs(x, Wq, Wk, Wv, b, h)
            in_maps.append(im)
    sh = NamedSharding(mesh, PartitionSpec("core"))
    concat_in = [
        jax.device_put(
            np.concatenate([np.asarray(in_maps[c][n]) for c in range(8)], axis=0), sh
        )
        for n in in_names
    ]

    def put_zeros():
        return [
            jax.device_put(np.zeros((8 * z.shape[0], *z.shape[1:]), z.dtype), sh)
            for z in zero_outs
        ]

    jax.block_until_ready(sharded(*concat_in, *put_zeros()))  # warmup
    times = {}
    for K in (4, iters, 4, iters):
        zs = [put_zeros() for _ in range(K)]
        jax.block_until_ready(zs)
        t0 = time.time()
        outs = [sharded(*concat_in, *z) for z in zs]
        jax.block_until_ready(outs)
        times[K] = min(times.get(K, 1e9), time.time() - t0)
    slope = (times[iters] - times[4]) / (iters - 4)
    return int(slope * 1e9)

